# revision 2
# baseline (speedup 1.0000x reference)
"""BatchSiren Trainium2 kernel.

B=2048 independent SIREN MLPs (2->32->32->3, sin activations, w0=30),
each evaluated on the same N=1024 coordinate grid.

Strategy (pure data parallel over 8 cores, 256 nets/core):
- 16 supergroups of 16 nets per core; within a supergroup, quad a holds
  nets {4a+b}. Activations live as [(member b, feature) partitions,
  points free]; quad a occupies free segment a of the layer-1/2 psum.
- L1: 16 nets at once via 32x32 tile_position (fp32, K=3 with the bias
  folded in as an augmented input row; coords tile rows are zero-padded).
- L2: one float32r block-diagonal [128x128] matmul per quad (f32r needs
  dst start_partition 0, but runs 4x faster than fp32's 2-pass mode).
  H1 is produced directly in float32r by the sin1 activation.
- L3: fp32 col-tiled matmuls with block-diagonal w3 produce the output
  point-major ([points, 4nets x 3ch]) so DMA writes 2KB runs/partition.
- sin(w0*z): weights pre-scaled by w0/2pi so matmuls produce arguments in
  CYCLE units; range-reduce with the magic-number rounding trick
  (ACT Identity+MAGIC bias for L1, DVE tensor_scalar for L2 where the
  per-net bias must be added pre-rounding), then ACT Sin with scale=-2pi.
  The Sin table only covers [-pi, pi].
"""
import numpy as np

import concourse.bacc as bacc
import concourse.bass as bass
import concourse.mybir as mybir
import concourse.tile as tile
from concourse import bass_utils

f32 = mybir.dt.float32
f32r = mybir.dt.float32r
AF = mybir.ActivationFunctionType
ALU = mybir.AluOpType

W0 = 30.0
MAGIC = float(1.5 * 2 ** 23)
TWO_PI = float(2.0 * np.pi)
N_CORES = 8
B, N, IN, H, OUT = 2048, 1024, 2, 32, 3
BPC = B // N_CORES        # 256 nets per core
SGS = BPC // 16           # 16 supergroups of 16 nets
NH = N // 2               # 512 points per half

_compiled = None


def _build_module():
    nc = bacc.Bacc("TRN2", target_bir_lowering=False, debug=False)

    d_w1 = nc.dram_tensor("w1aug", [4, 3, 128 * SGS], f32, kind="ExternalInput")
    d_w2 = nc.dram_tensor("w2blk", [SGS, 4, 128, 128], f32r, kind="ExternalInput")
    d_w3 = nc.dram_tensor("w3blk", [4, 32, 48 * SGS], f32, kind="ExternalInput")
    d_sm = nc.dram_tensor("smalls", [128, 9 * SGS], f32, kind="ExternalInput")
    d_c = nc.dram_tensor("coords", [4, 3, N], f32, kind="ExternalInput")
    d_out = nc.dram_tensor("out", [SGS, 2, 48, NH], f32, kind="ExternalOutput")

    with tile.TileContext(nc) as tc:
        with tc.tile_pool(name="const", bufs=1) as cp, \
             tc.tile_pool(name="acts", bufs=2) as ap, \
             tc.tile_pool(name="outp", bufs=3) as op_, \
             tc.tile_pool(name="psA", bufs=3, space="PSUM") as psA, \
             tc.tile_pool(name="psC", bufs=2, space="PSUM") as psC:

            # ---- persistent constants ----
            w1sb = cp.tile([128, 128 * SGS], f32, tag="w1")
            for a in range(4):
                nc.sync.dma_start(w1sb[32 * a:32 * a + 3, :], d_w1[a])
            w2sb = cp.tile([128, 512 * SGS], f32r, tag="w2")
            for sg in range(SGS):
                for a in range(4):
                    nc.sync.dma_start(
                        w2sb[:, 512 * sg + 128 * a:512 * sg + 128 * a + 128],
                        d_w2[sg, a])
            w3sb = cp.tile([128, 48 * SGS], f32, tag="w3")
            for a in range(4):
                nc.sync.dma_start(w3sb[32 * a:32 * a + 32, :], d_w3[a])
            smalls = cp.tile([128, 9 * SGS], f32, tag="sm")
            nc.sync.dma_start(smalls[:], d_sm[:])
            c4 = cp.tile([128, N], f32, tag="c4")
            nc.vector.memset(c4[:], 0.0)  # rows 3-31 of each group MUST be 0
            for a in range(4):
                nc.sync.dma_start(c4[32 * a:32 * a + 3, :], d_c[a])
            magic = cp.tile([128, 1], f32, tag="mg")
            nc.vector.memset(magic[:], MAGIC)

            for sg in range(SGS):
                Q2 = ap.tile([128, 4096], f32, tag="Q2")
                H2 = ap.tile([128, 4096], f32, tag="H2")
                for h in range(2):
                    # ---- layer 1: 16 nets as 32x32 tiles, K=3 (w,b aug) ----
                    # PZ1 split in two [128,1024] psum tiles: quads {0,1},{2,3}
                    T1 = ap.tile([128, 2048], f32, tag="T1")
                    Q1 = ap.tile([128, 2048], f32, tag="Q1")
                    for g in range(2):
                        PZ1 = psA.tile([128, 1024], f32, tag="P")
                        for a in (2 * g, 2 * g + 1):
                            la = a % 2
                            for b in range(4):
                                nc.tensor.matmul(
                                    out=PZ1[32 * b:32 * b + 32,
                                            512 * la:512 * la + 512],
                                    lhsT=w1sb[32 * a:32 * a + 3,
                                              128 * sg + 32 * b:128 * sg + 32 * b + 32],
                                    rhs=c4[32 * a:32 * a + 3, NH * h:NH * h + NH],
                                    start=True, stop=True,
                                    tile_position=(32 * a, 32 * b))
                        sl = slice(1024 * g, 1024 * g + 1024)
                        nc.scalar.activation(T1[:, sl], PZ1[:], AF.Identity,
                                             bias=magic[:], scale=1.0)
                        nc.vector.scalar_tensor_tensor(
                            Q1[:, sl], T1[:, sl], MAGIC, PZ1[:],
                            ALU.subtract, ALU.subtract)
                    H1 = ap.tile([128, 2048], f32r, tag="H1")
                    nc.scalar.activation(H1[:], Q1[:], AF.Sin,
                                         bias=0.0, scale=-TWO_PI)

                    # ---- layer 2: f32r block-diag, one MM per quad a ----
                    T2 = ap.tile([128, 2048], f32, tag="T2")
                    for g in range(2):
                        PZ2 = psA.tile([128, 1024], f32, tag="P")
                        for a in (2 * g, 2 * g + 1):
                            la = a % 2
                            nc.tensor.matmul(
                                out=PZ2[:, 512 * la:512 * la + 512],
                                lhsT=w2sb[:, 512 * sg + 128 * a:
                                          512 * sg + 128 * a + 128],
                                rhs=H1[:, 512 * a:512 * a + 512],
                                start=True, stop=True)
                            # t2 = (z + b2cyc) + MAGIC (bias pre-round)
                            nc.vector.tensor_scalar(
                                T2[:, 512 * a:512 * a + 512],
                                PZ2[:, 512 * la:512 * la + 512],
                                smalls[:, 9 * sg + a:9 * sg + a + 1], MAGIC,
                                ALU.add, ALU.add)
                        # q2 = (t2 - MAGIC) - z, scattered into Q2 as (a, h, n)
                        t2v = T2[:].rearrange("p (a n) -> p a n", a=4)[
                            :, 2 * g:2 * g + 2, :]
                        q2v = Q2[:].rearrange("p (a g2 n) -> p a g2 n",
                                              a=4, g2=2)[:, 2 * g:2 * g + 2, h, :]
                        z2v = PZ2[:].rearrange("p (a n) -> p a n", a=2)
                        nc.vector.scalar_tensor_tensor(
                            q2v, t2v, MAGIC, z2v, ALU.subtract, ALU.subtract)

                # ---- sin2, batched over both halves per quad (shared bias) ----
                for a in range(4):
                    nc.scalar.activation(
                        H2[:, 1024 * a:1024 * a + 1024],
                        Q2[:, 1024 * a:1024 * a + 1024],
                        AF.Sin,
                        bias=smalls[:, 9 * sg + 4 + a:9 * sg + 4 + a + 1],
                        scale=-TWO_PI)

                # ---- layer 3: col-tiled fp32, block-diag w3, point-major ----
                for h in range(2):
                    PC = psC.tile([128, 512], f32, tag="C")
                    for a in range(4):
                        nc.tensor.matmul(
                            out=PC[32 * a:32 * a + 12, :],
                            lhsT=w3sb[:, 48 * sg + 12 * a:48 * sg + 12 * a + 12],
                            rhs=H2[:, 1024 * a + 512 * h:1024 * a + 512 * h + 512],
                            start=True, stop=True,
                            tile_position=(0, 32 * a))
                    OT = op_.tile([128, 512], f32, tag="OT")
                    nc.scalar.activation(OT[:], PC[:], AF.Identity,
                                         bias=smalls[:, 9 * sg + 8:9 * sg + 9],
                                         scale=1.0)
                    for a in range(4):
                        nc.sync.dma_start(
                            d_out[sg, h, 12 * a:12 * a + 12, :],
                            OT[32 * a:32 * a + 12, :])

    nc.compile()
    return nc


def _prep_core_inputs(w1, b1, w2, b2, w3, b3, coords, core):
    s = np.float32(W0 / TWO_PI)
    B0 = core * BPC
    sl = slice(B0, B0 + BPC)

    # net (sg, a, b) = batch B0 + 16sg + 4a + b
    w1c = w1[sl, :, :, 0].reshape(SGS, 4, 4, IN, H)
    b1c = b1[sl, :, 0].reshape(SGS, 4, 4, H)
    aug = np.concatenate([w1c, b1c[:, :, :, None, :]], axis=3) * s  # [sg,a,b,3,32]
    w1aug = np.ascontiguousarray(
        aug.transpose(1, 3, 0, 2, 4).reshape(4, 3, SGS * 128)).astype(np.float32)

    # L2 block-diag per (sg, a): [128,128], block b = w2[net(sg,a,b)] * s
    w2c = (w2[sl, :, :, 0] * s).reshape(SGS, 4, 4, H, H)  # [sg,a,b,i,o]
    w2blk = np.zeros((SGS, 4, 128, 128), np.float32)
    for b in range(4):
        w2blk[:, :, 32 * b:32 * b + 32, 32 * b:32 * b + 32] = w2c[:, :, b]

    # L3 block-diag per (sg, a): [128, 12], [32b+i, 3b+c] = w3[net(sg,a,b), i, c]
    # stored as [4(row-group b), 32(i), sg, 12a + 3b' + c] with zeros off-diag
    w3c = w3[sl, :, :, 0].reshape(SGS, 4, 4, H, OUT)  # [sg,a,b,i,c]
    blk = np.zeros((SGS, 4, 4, H, 4, OUT), np.float32)  # [sg,a,b,i,b',c]
    for b in range(4):
        blk[:, :, b, :, b, :] = w3c[:, :, b]
    # free index inside sg block: 12*a + 3*b' + c ; partition 32*b + i
    w3blk = np.ascontiguousarray(
        blk.transpose(2, 3, 0, 1, 4, 5).reshape(4, 32, SGS * 48)).astype(np.float32)

    b2c = b2[sl, :, 0].reshape(SGS, 4, 4, H)  # [sg,a,b,o]
    b3c = b3[sl, :, 0].reshape(SGS, 4, 4, OUT)  # [sg,a,b,c]
    smalls = np.zeros((128, SGS, 9), np.float32)
    p = np.arange(128)
    b_idx, o_idx = p // 32, p % 32
    for a in range(4):
        # partition 32b+o ; quad a -> cols a (cycles) / 4+a (radians)
        smalls[:, :, a] = (b2c[:, a, b_idx, o_idx] * s).T
        smalls[:, :, 4 + a] = (b2c[:, a, b_idx, o_idx] * np.float32(W0)).T
    # b3: partition 32a + 3b + c
    a_idx, m_idx = p // 32, p % 32
    b3v, c3v = m_idx // 3, m_idx % 3
    for pi in range(128):
        if m_idx[pi] < 12:
            smalls[pi, :, 8] = b3c[:, a_idx[pi], b3v[pi], c3v[pi]]
    smalls = np.ascontiguousarray(smalls.reshape(128, SGS * 9))

    ch = np.zeros((4, 3, N), np.float32)
    ch[:, :IN, :] = coords.T[None, :, :]
    ch[:, IN, :] = 1.0

    return {"w1aug": w1aug, "w2blk": w2blk, "w3blk": w3blk,
            "smalls": smalls, "coords": ch}


def _unshard(res_list):
    outs = []
    for r in res_list:
        o = r["out"].reshape(SGS, 2, 4, 4, OUT, NH)      # [sg,h,a,b,c,n]
        o = o.transpose(0, 2, 3, 1, 5, 4)                # [sg,a,b,h,n,c]
        outs.append(np.ascontiguousarray(o.reshape(BPC, N, OUT)))
    return np.concatenate(outs, axis=0)


def _run(inputs, trace=False, trace_kwargs=None):
    global _compiled
    if _compiled is None:
        _compiled = _build_module()
    nc = _compiled
    arrs = {k: np.asarray(v, dtype=np.float32) for k, v in inputs.items()}
    in_maps = [_prep_core_inputs(arrs["w1"], arrs["b1"], arrs["w2"], arrs["b2"],
                                 arrs["w3"], arrs["b3"], arrs["coords"], c)
               for c in range(N_CORES)]
    kw = {}
    if trace:
        kw["trace"] = True
        if trace_kwargs:
            kw.update(trace_kwargs)
    res = bass_utils.run_bass_kernel_spmd(nc, in_maps, core_ids=list(range(N_CORES)),
                                          **kw)
    out = _unshard(res.results)
    return out, res


def kernel(**inputs):
    out, _ = _run(inputs, trace=False)
    return out
